# revision 123
# baseline (speedup 1.0000x reference)
"""Trainium2 Bass kernel for causal multi-head attention with RoPE + register tokens.

Problem (nn_Attention_38293928411140):
  B=1, S=4096, HIDDEN=512, 8 heads x head_dim 64, causal SDPA, RoPE applied to
  positions >= num_registers (cos/sin indexed by position - num_registers), fp32.
  out = softmax(causal(QK^T/8)) V followed by a Wo projection.

Sharding: tensor-parallel over heads -- one head per NeuronCore (8 heads, 8 cores).

Per-core kernel (fused causal pipeline). The softmax exp stream on the
Activation engine is the throughput floor (~0.83 ns/column over the causal
score area), so the kernel is organized to keep every other engine underneath
it and the Act queue exp-only:
  - X^T is passed pre-transposed from the host (layout packing), so Q/K/V
    projections read SBUF-resident X^T chunks directly -- no on-chip
    transpose pass.
  - RoPE's rotate_half is a partition shuffle: the PSUM projection stages
    once into a bf16 SBUF copy, then all-bf16 muls against host-packed
    cos / signed-sin tables run the DVE in its 2x mode (table rows arranged
    to satisfy walrus's equal-input-base-partition rule).
  - roped Q^T/K^T are stored fp8e4 in DoubleRow layout [32, 2, S]; the
    scores^T [k, q] chunks contract all 64 head dims in one half-rate PE
    pass (0.5 cycles/column; fp8 q/k costs ~5e-3 relative error, CPU-
    validated -- fp8 probs or V would cost ~2.5e-2 and are kept bf16).
  - exp on ScalarE reads PSUM in 1024-wide strides and writes bf16 probs
    (softmax max-shift skipped -- exact by shift invariance, scores are
    bounded); the diagonal chunk pair exps only its causally-reachable
    column wedges.  Causal mask via gpsimd affine_select after exp; O^T
    accumulates in PSUM from bf16 probs x bf16 V-tiles with a ones-column
    appended to V producing row-sums for free.
  - supertile c of attention only needs chunks <= c: the X^T load for c+2
    and the projection/rope for c+1 are interleaved into supertile c, and
    the first score/exp group of supertile c+1 is emitted before c's tail
    so Act never idles across boundaries.
  - delayed normalization: each supertile ships unnormalized O^T plus a
    bf16 rowsum reciprocal row through the AllToAll head-exchange; after
    the exchange a ones-selector matmul broadcasts the reciprocals and one
    bf16 mul normalizes each head-pair block right before the bf16 Wo
    projection (ci-major accumulation, early blocks running behind the
    last supertile).
Host side only packs layouts (per-head weight slices, transposes, signed
trig tables, bf16 casts of Wo, identity/selector constants) and concatenates
the 8 output shards.

A post-scheduling pass hoists extra semaphore waits onto sequencer no-ops
because this walrus build rejects instructions with more than one sync wait.
"""
import math
import numpy as np
import ml_dtypes

import concourse.bass as bass
import concourse.mybir as mybir
import concourse.tile as tile
from concourse.bass_utils import run_bass_kernel_spmd

F32 = mybir.dt.float32
F32R = mybir.dt.float32r
BF16 = mybir.dt.bfloat16
FP8 = mybir.dt.float8e4

HIDDEN = 512
NHEADS = 8
HD = 64
NCORES = 8
SCALE = 1.0 / math.sqrt(HD)

_PROGRAM_CACHE = {}


def _split_matmul_waits(nc):
    """Walrus's CoreV3 codegen rejects instructions carrying more than one sync
    wait ('Too many sync wait commands', e.g. Matmult LW_STRUCT and Drain).
    Hoist all but one wait onto same-engine sequencer no-ops inserted right
    before the instruction -- semantically identical (the sequencer satisfies
    the waits in program order before issuing it)."""
    import bass_rust
    for f in nc.m.functions:
        for blk in f.blocks:
            out = []
            for inst in blk.instructions:
                si = getattr(inst, "sync_info", None)
                eng = getattr(inst, "engine", None)
                if si is not None and eng is not None and len(si.on_wait) > 1:
                    waits = list(si.on_wait)
                    for k, w in enumerate(waits[:-1]):
                        nop = bass_rust.InstNoOp(
                            name=f"{inst.name}-hw{k}",
                            engine=eng,
                            text_hint="hoisted-wait",
                            sync_info=mybir.SyncInfo(on_wait=[w], on_update=[]),
                        )
                        out.append(nop)
                    inst.sync_info = mybir.SyncInfo(
                        on_wait=[waits[-1]], on_update=list(si.on_update))
                out.append(inst)
            blk.instructions = out


def build_program(S=4096, hoist=True, repeat=1, mock_cc=False, hw_loop=0,
                  fast_mm=True):
    """Build the SPMD Bass program (same NEFF on all 8 cores).

    Fused causal pipeline: supertile c of the attention only needs Q/K/V
    chunks <= c, so chunk prep (X^T load, projections, rope, V layout) for
    chunk c+1 is emitted interleaved with attention supertile c and overlaps
    it across engines."""
    assert S % 512 == 0
    W = 512                      # q-supertile width == s-chunk width
    NSUP = S // W
    NST = S // 128
    SHARD = S // NCORES

    nc = bass.Bass("TRN2", target_bir_lowering=False, debug=False,
                   num_devices=NCORES)

    xT = nc.dram_tensor("xT", [HIDDEN, S], F32R, kind="ExternalInput").ap()
    wqkT = nc.dram_tensor("wqkT", [HIDDEN, 2 * HD], F32R, kind="ExternalInput").ap()
    wvT = nc.dram_tensor("wvT", [HIDDEN, HD], F32R, kind="ExternalInput").ap()
    woT = nc.dram_tensor("woT", [HIDDEN, HIDDEN], BF16, kind="ExternalInput").ap()
    # trigA = cos^T duplicated on rows 0:64 and 64:128 (q and k halves);
    # trigB rows: 0:32=-sin^T[0:32], 32:64=sin^T[32:64], 64:96=-sin^T[0:32],
    # 96:128=sin^T[32:64] -- arranged so each rope mul's SBUF operand base
    # partition matches its output tile base partition.
    trigA = nc.dram_tensor("trigA", [128, S], BF16, kind="ExternalInput").ap()
    trigB = nc.dram_tensor("trigB", [128, S], BF16, kind="ExternalInput").ap()
    identb_in = nc.dram_tensor("identb", [128, 128], BF16, kind="ExternalInput").ap()
    out_shard = nc.dram_tensor("out_shard", [SHARD, HIDDEN], F32,
                               kind="ExternalOutput").ap()

    # exchanged rows: 64 unnormalized O^T dims + the rowsum-reciprocal row;
    # normalization happens after the exchange (delayed normalization), so
    # the per-supertile tail is just a PSUM->SBUF copy + DMA.
    a2a_in = nc.dram_tensor("a2a_in", [NCORES, HD + 1, SHARD], BF16)
    a2a_out = nc.dram_tensor("a2a_out", [NCORES, HD + 1, SHARD], BF16)
    # ones-selector broadcasting reciprocal row p//64 onto the 128 partition
    # rows of an otall block: sel2[i, p] = 1 iff p // 64 == i
    sel2 = nc.dram_tensor("sel2", [2, 128], BF16, kind="ExternalInput").ap()

    Exp = mybir.ActivationFunctionType.Exp

    with tile.TileContext(nc) as tc:
      with tc.tile_pool(name="persist", bufs=1) as pp:
        identb = pp.tile([128, 128], BF16)
        # roped Q^T/K^T in fp8 DoubleRow layout: [32, 2, S], dim1 = head-dim
        # half, so the scores matmul contracts all 64 dims in one half-rate
        # pass (CPU-validated: fp8 q/k costs ~5e-3 extra relative error)
        qt = pp.tile([32, 2, S], FP8, tag="qt")
        kt = pp.tile([32, 2, S], FP8, tag="kt")
        vext = pp.tile([128, NST * 65], BF16, tag="vext")  # V tiles + ones col
        ones_b = pp.tile([128, 32], F32, tag="ones32")
        sel_sb = pp.tile([2, 128], BF16, tag="sel2")
        wqk_sb = pp.tile([128, 4 * 128], F32R, tag="wqk")
        wv_sb = pp.tile([128, 4 * 64], F32R, tag="wv")
        trigA_sb = pp.tile([128, S], BF16, tag="trigA")
        trigB_sb = pp.tile([128, S], BF16, tag="trigB")
        wo_sb = pp.tile([128, 4 * 512], BF16, tag="wo")

        # Weight loads are single batched DMAs (each DMA pays a ~500ns
        # minimum).  Trig tables preload once: the chunk-0 slices small and
        # early on Pool, the rest as two big strided DMAs on SP.  Bulky
        # wo/identb loads ride the Activation queue, idle until the first exp
        # ~5us in; a throwaway exp there warms the Act function table.
        nc.gpsimd.memset(ones_b[:], 1.0)
        nc.gpsimd.tensor_copy(
            vext[:].rearrange("p (t c) -> p t c", c=65)[:, :, 64],
            ones_b[:, 0:NST])
        warm = pp.tile([1, 32], F32, tag="warm")

        def load_weights():
            # emitted after the chunk-0/1 X^T loads so they don't push the
            # first projection past the Pool DMA queue
            nc.gpsimd.dma_start(
                wqk_sb[:].rearrange("p (c n) -> p c n", n=128),
                wqkT.rearrange("(c p) n -> p c n", p=128))
            nc.gpsimd.dma_start(
                wv_sb[:].rearrange("p (c n) -> p c n", n=64),
                wvT.rearrange("(c p) n -> p c n", p=128))
            nc.gpsimd.dma_start(sel_sb[:], sel2)
            nc.gpsimd.dma_start(identb[:], identb_in)

        import contextlib
        loop_cm = tc.For_i(0, hw_loop, 1) if hw_loop else contextlib.nullcontext()
        with loop_cm:
          for _rep in range(repeat):
            with tc.tile_pool(name="attn", bufs=1) as pa, \
                 tc.tile_pool(name="xtc", bufs=2) as pxtc, \
                 tc.tile_pool(name="vtc", bufs=2) as pvtc, \
                 tc.tile_pool(name="rope", bufs=2) as prt, \
                 tc.tile_pool(name="pt", bufs=3) as ppt, \
                 tc.tile_pool(name="psc", bufs=2, space="PSUM") as psc, \
                 tc.tile_pool(name="pprep", bufs=2, space="PSUM") as pprep, \
                 tc.tile_pool(name="psot", bufs=2, space="PSUM") as psot:
                ot = pa.tile([64, S], BF16, tag="ot")
                rsb = pa.tile([1, S], BF16, tag="rsb")

                xtc_by_c = {}

                def prep_a(c):
                    cs = slice(c * 512, (c + 1) * 512)
                    xg = pxtc.tile([128, 4, 512], F32R, tag="xin", name="xg")
                    # per-chunk trig slices on SP: each lands two supertiles
                    # before its rope consumer, spreading the load evenly
                    nc.sync.dma_start(trigA_sb[:, cs], trigA[:, cs])
                    nc.sync.dma_start(trigB_sb[:, cs], trigB[:, cs])
                    engs = (nc.sync, nc.sync, nc.gpsimd,
                            nc.scalar if c < 2 else nc.gpsimd)
                    for hj in range(4):
                        engs[hj].dma_start(xg[:, hj, :],
                                           xT[hj * 128:(hj + 1) * 128, cs])
                    xtc_by_c[c] = xg

                def prep_b(c):
                    cs = slice(c * 512, (c + 1) * 512)
                    xg = xtc_by_c[c]
                    pqk = pprep.tile([128, 512], F32, tag="prep", name="pqk")
                    for hj in range(4):
                        nc.tensor.matmul(
                            pqk[:], lhsT=wqk_sb[:, hj * 128:(hj + 1) * 128],
                            rhs=xg[:, hj, :], start=(hj == 0), stop=(hj == 3))
                    pk = prt.tile([128, 512], BF16, tag="pk", name="pk")
                    nc.vector.tensor_copy(pk[:], pqk[:])
                    t1 = prt.tile([128, 512], BF16, tag="t1", name="t1")
                    t2 = prt.tile([128, 512], BF16, tag="t2", name="t2")
                    # t1 = q|k elementwise cos (one full-width mul);
                    # t2 = rotate_half(q|k) . sin via partition-offset reads
                    # of the bf16 SBUF staging copy -- all-bf16 SBUF operands
                    # run the DVE in its 2x mode.  trigB rows are arranged to
                    # match each mul's SOURCE partitions (walrus requires
                    # equal input base partitions; the output may differ).
                    # The q half completes first -- supertile s+1's earliest
                    # score groups need only qt(s+1) plus already-roped K.
                    nc.vector.tensor_mul(t1[:, :], pk[:, :], trigA_sb[:, cs])
                    nc.vector.tensor_mul(t2[0:32, :], pk[32:64, :],
                                         trigB_sb[32:64, cs])
                    nc.vector.tensor_mul(t2[32:64, :], pk[0:32, :],
                                         trigB_sb[0:32, cs])
                    nc.gpsimd.tensor_add(qt[:, 0, cs], t1[0:32, :], t2[0:32, :])
                    nc.gpsimd.tensor_add(qt[:, 1, cs], t1[32:64, :], t2[32:64, :])
                    nc.vector.tensor_mul(t2[64:96, :], pk[96:128, :],
                                         trigB_sb[96:128, cs])
                    nc.vector.tensor_mul(t2[96:128, :], pk[64:96, :],
                                         trigB_sb[64:96, cs])
                    nc.gpsimd.tensor_add(kt[:, 0, cs], t1[64:96, :], t2[64:96, :])
                    nc.gpsimd.tensor_add(kt[:, 1, cs], t1[96:128, :], t2[96:128, :])

                def prep_c(c):
                    xg = xtc_by_c.pop(c)
                    pv = pprep.tile([64, 512], F32, tag="prep", name="pv")
                    for hj in range(4):
                        nc.tensor.matmul(
                            pv[:], lhsT=wv_sb[:, hj * 64:(hj + 1) * 64],
                            rhs=xg[:, hj, :], start=(hj == 0), stop=(hj == 3))
                    vtc = pvtc.tile([64, 512], BF16, tag="vtc", name="vtc")
                    nc.vector.tensor_copy(vtc[:], pv[:])
                    pst = pprep.tile([128, 256], BF16, tag="prep", name="pst")
                    for k in range(4):
                        nc.tensor.transpose(
                            pst[:, k * 64:(k + 1) * 64],
                            vtc[:, k * 128:(k + 1) * 128],
                            identb[0:64, 0:64])
                    nc.vector.tensor_copy(
                        vext[:].rearrange("p (t c) -> p t c", c=65)[
                            :, 4 * c:4 * c + 4, 0:64],
                        pst[:].rearrange("p (t c) -> p t c", c=64))

                def attn_groups(sup, otp, glo, ghi):
                    qs = slice(sup * 512, (sup + 1) * 512)
                    npairs = (sup + 1) * 4
                    for g in range(glo, ghi):
                        pg = min(2, npairs - g * 2)
                        sp = psc.tile([128, pg * 512], F32, tag="sc", name="sp")
                        for p in range(pg):
                            kp = g * 2 + p
                            o = p * 512
                            nc.tensor.matmul(
                                sp[:, o:o + 512],
                                lhsT=kt[:, :, kp * 128:(kp + 1) * 128],
                                rhs=qt[:, :, qs], start=True, stop=True,
                                perf_mode=mybir.MatmulPerfMode.DoubleRow)
                        ptile = ppt.tile([128, pg * 512], BF16, tag="pt",
                                         name="ptile")
                        if g == ngroups - 1:
                            # diagonal pair (kp = 4*sup+2, 4*sup+3): only
                            # columns [256:512] / [896:1024] are causally
                            # reachable -- exp just those, zero-fill the rest
                            # (bitcast: bf16 memset fails the ISA check)
                            nc.gpsimd.memset(ptile[:, 0:256].bitcast(F32), 0.0)
                            nc.gpsimd.memset(ptile[:, 512:896].bitcast(F32),
                                             0.0)
                            nc.scalar.activation(ptile[:, 256:512],
                                                 sp[:, 256:512], Exp,
                                                 scale=SCALE)
                            nc.scalar.activation(ptile[:, 896:1024],
                                                 sp[:, 896:1024], Exp,
                                                 scale=SCALE)
                            nc.gpsimd.affine_select(
                                out=ptile[:, 256:512],
                                in_=ptile[:, 256:512], pattern=[[1, 256]],
                                compare_op=mybir.AluOpType.is_ge, fill=0.0,
                                base=0, channel_multiplier=-1)
                            nc.gpsimd.affine_select(
                                out=ptile[:, 896:1024],
                                in_=ptile[:, 896:1024], pattern=[[1, 128]],
                                compare_op=mybir.AluOpType.is_ge, fill=0.0,
                                base=0, channel_multiplier=-1)
                        else:
                            nc.scalar.activation(ptile[:, 0:pg * 512],
                                                 sp[:, 0:pg * 512], Exp,
                                                 scale=SCALE)
                            for p in range(pg):
                                kp = g * 2 + p
                                if kp >= sup * 4:
                                    nc.gpsimd.affine_select(
                                        out=ptile[:, p * 512:(p + 1) * 512],
                                        in_=ptile[:, p * 512:(p + 1) * 512],
                                        pattern=[[1, 512]],
                                        compare_op=mybir.AluOpType.is_ge,
                                        fill=0.0,
                                        base=sup * 512 - kp * 128,
                                        channel_multiplier=-1)
                        for p in range(pg):
                            kp = g * 2 + p
                            nc.tensor.matmul(
                                otp[:], lhsT=vext[:, kp * 65:kp * 65 + 65],
                                rhs=ptile[:, p * 512:(p + 1) * 512],
                                start=(kp == 0), stop=(kp == npairs - 1))

                def attn_tail(sup, otp, mock_xchg):
                    qs = slice(sup * 512, (sup + 1) * 512)
                    # delayed normalization: ship unnormalized O^T plus the
                    # rowsum reciprocal; nothing downstream in the loop waits
                    # on this chain, so supertile boundaries stay fluid.
                    with nc.allow_low_precision(
                            reason="bf16 rowsum reciprocal travels the bf16 "
                                   "head-exchange; applied to bf16 data"):
                        nc.vector.reciprocal(rsb[0:1, qs], otp[64:65, :])
                    if mock_xchg and sup == NSUP - 1:
                        # identity exchange reads the last slice straight
                        # from PSUM/rsb; skip its a2a staging entirely
                        return
                    nc.vector.tensor_copy(ot[:, qs], otp[0:64, :])
                    j = sup
                    nc.gpsimd.dma_start(a2a_in.ap()[j][64:65, :],
                                        rsb[0:1, j * SHARD:(j + 1) * SHARD])
                    nc.sync.dma_start(a2a_in.ap()[j][0:64, :],
                                      ot[:, j * SHARD:(j + 1) * SHARD])
                    if mock_xchg and sup < NSUP - 1:
                        # mock exchange: per-destination copy issued as soon
                        # as this supertile's slice is written.  (The real
                        # path runs one AllToAll after the loop.)
                        nc.sync.dma_start(a2a_out.ap()[j], a2a_in.ap()[j])

                otall = pa.tile([128, 4 * SHARD], BF16, tag="otall")
                otn = pa.tile([128, 4 * SHARD], BF16, tag="otn")
                recs = pa.tile([2, 4 * SHARD], BF16, tag="recs")

                def gather_ci(ci, src):
                    # pull heads 2ci / 2ci+1 (+ their reciprocal rows) of this
                    # core's shard out of the exchange buffer, one DMA queue
                    # per transfer (the tile scheduler tracks the DRAM deps)
                    ob = slice(ci * SHARD, (ci + 1) * SHARD)
                    nc.sync.dma_start(otall[0:64, ob], src.ap()[2 * ci][0:64, :])
                    nc.gpsimd.dma_start(otall[64:128, ob],
                                        src.ap()[2 * ci + 1][0:64, :])
                    nc.scalar.dma_start(recs[:, ob],
                                        src.ap()[2 * ci:2 * ci + 2, 64, :])

                def normalize_ci(ci):
                    # scale[p, s] = recs[p // 64, s] broadcast via the ones
                    # selector, then one bf16 mul normalizes the block.
                    # ci=3 draws from psot (its slot frees at tail(6)); a
                    # pprep slot would deadlock against the po tiles.
                    ob = slice(ci * SHARD, (ci + 1) * SHARD)
                    pool = pprep if ci < 3 else psot
                    tag = "prep" if ci < 3 else "otp"
                    sc = pool.tile([128, 512], F32, tag=tag, name="scale")
                    nc.tensor.matmul(sc[:], lhsT=sel_sb[:],
                                     rhs=recs[:, ob], start=True, stop=True)
                    nc.vector.tensor_mul(otn[:, ob], otall[:, ob], sc[:])

                # xg DMAs run two supertiles ahead, projections/rope one
                # ahead: the load for chunk c+2 and the compute for chunk c+1
                # are both interleaved into supertile c's attention.  In mock
                # mode the exchange copies, gather-loads and normalization of
                # early head-pairs also drain per-supertile instead of in the
                # tail (the real path needs the single AllToAll done first).
                sc3_holder = []
                prep_a(0)
                prep_a(1)
                load_weights()
                # Act function-table warm-up rides after the chunk-0/1 hj3
                # loads; exp tables are resident before the first real exp
                nc.scalar.activation(warm[:], ones_b[0:1, :],
                                     mybir.ActivationFunctionType.Exp)
                prep_b(0)
                prep_b(1)
                prep_c(0)
                otp_next = None
                for sup in range(NSUP):
                    ngroups = ((sup + 1) * 4 + 1) // 2
                    if otp_next is not None:
                        otp = otp_next
                        glo = 1      # group 0 was emitted with the last sup
                    else:
                        otp = psot.tile([65, 512], F32, tag="otp", name="otp")
                        glo = 0
                    h = max(1, ngroups // 2)
                    attn_groups(sup, otp, glo, h)
                    if sup + 2 < NSUP:
                        prep_a(sup + 2)
                    if sup + 1 < NSUP:
                        prep_c(sup + 1)
                    attn_groups(sup, otp, h, ngroups)
                    if sup + 1 < NSUP:
                        # bridge the supertile boundary: the next supertile's
                        # first score/exp/PV group keeps Act busy while this
                        # supertile's tail chain drains
                        otp_next = psot.tile([65, 512], F32, tag="otp",
                                             name="otp")
                        attn_groups(sup + 1, otp_next, 0, 1)
                    else:
                        otp_next = None
                    if sup == NSUP - 1 and mock_cc:
                        for ci in range(3):
                            normalize_ci(ci)
                        # final-slice pieces that depend only on supertile 6:
                        # head 6's block copy and its reciprocal broadcast
                        ob3 = slice(3 * SHARD, 4 * SHARD)
                        nc.gpsimd.tensor_copy(otall[0:64, ob3],
                                              ot[:, 6 * 512:7 * 512])
                        sc3 = psot.tile([128, 512], F32, tag="otp",
                                        name="scale")
                        nc.tensor.matmul(sc3[0:64, :],
                                         lhsT=sel_sb[0:1, 0:64],
                                         rhs=rsb[0:1, 6 * 512:7 * 512],
                                         start=True, stop=True)
                        sc3_holder.append(sc3)
                    attn_tail(sup, otp, mock_cc)
                    if sup + 2 < NSUP:
                        prep_b(sup + 2)
                    if sup == 5:
                        # wo needed only by the final projection; SP is quiet
                        # by now (no more prep_a loads after supertile 5)
                        nc.sync.dma_start(
                            wo_sb[:].rearrange("p (c n) -> p c n", n=512),
                            woT.rearrange("(c p) n -> p c n", p=128))
                    if mock_cc and sup % 2 == 1 and sup < NSUP - 1:
                        gather_ci((sup - 1) // 2, a2a_out)

                # ---- exchange heads, output projection ----
                with tc.tile_pool(name="fout", bufs=4) as pfo:
                    # ci-major Wo accumulation over four live PSUM tiles (two
                    # from psc, two from the now-idle pprep): the ci<3 passes
                    # are emitted before the last exchange slice is even
                    # gathered, so they run warm behind the last supertile;
                    # only ci=3 chains on the final slice.  The pos tiles
                    # must allocate AFTER the scale tiles of the same pools
                    # or the pool rotation deadlocks.
                    pos = []

                    def alloc_pos():
                        pos.extend([
                            psc.tile([128, 512], F32, tag="sc", name="po"),
                            psc.tile([128, 512], F32, tag="sc", name="po"),
                            pprep.tile([128, 512], F32, tag="prep", name="po"),
                            pprep.tile([128, 512], F32, tag="prep", name="po")])

                    def po_pass(ci):
                        for ss in range(SHARD // 128):
                            nc.tensor.matmul(
                                pos[ss][:],
                                lhsT=otn[:, ci * SHARD + ss * 128:
                                         ci * SHARD + (ss + 1) * 128],
                                rhs=wo_sb[:, ci * 512:(ci + 1) * 512],
                                start=(ci == 0), stop=(ci == 3))

                    if mock_cc:
                        alloc_pos()
                        for ci in range(3):
                            po_pass(ci)
                        # identity exchange: the final slice pair (heads 6,7)
                        # is this core's own ot/rsb data -- read it from SBUF
                        # instead of paying two more DRAM round-trip
                        # latencies.  Head 6's half was emitted at supertile
                        # 7; only head 7's pieces chain on the last tail.
                        ob = slice(3 * SHARD, 4 * SHARD)
                        sc = sc3_holder.pop()
                        # head 7's block straight out of PSUM on DVE (skips
                        # waiting on the a2a staging copy of ot)
                        nc.vector.tensor_copy(otall[64:128, ob], otp[0:64, :])
                        nc.tensor.matmul(sc[64:128, :],
                                         lhsT=sel_sb[0:1, 0:64],
                                         rhs=rsb[0:1, 7 * 512:8 * 512],
                                         start=True, stop=True)
                        # per-ss normalization slices let each Wo column
                        # block fire as soon as its slice is scaled
                        for ss in range(SHARD // 128):
                            sl = slice(3 * SHARD + ss * 128,
                                       3 * SHARD + (ss + 1) * 128)
                            nc.vector.tensor_mul(otn[:, sl], otall[:, sl],
                                                 sc[:, ss * 128:(ss + 1) * 128])
                            nc.tensor.matmul(
                                pos[ss][:], lhsT=otn[:, sl],
                                rhs=wo_sb[:, 3 * 512:4 * 512],
                                start=False, stop=True)
                            osb = pfo.tile([128, 512], F32, tag="osb",
                                           name="osb")
                            nc.vector.tensor_copy(osb[:], pos[ss][:])
                            eng = (nc.sync, nc.gpsimd, nc.scalar, nc.sync)[ss]
                            eng.dma_start(
                                out_shard[ss * 128:(ss + 1) * 128, :], osb[:])
                    else:
                        nc.gpsimd.collective_compute(
                            "AllToAll", mybir.AluOpType.bypass,
                            replica_groups=[list(range(NCORES))],
                            ins=[a2a_in.ap()], outs=[a2a_out.ap()])
                        for ci in range(4):
                            gather_ci(ci, a2a_out)
                        for ci in range(3):
                            normalize_ci(ci)
                        alloc_pos()
                        for ci in range(3):
                            po_pass(ci)
                        normalize_ci(3)
                        for ss in range(SHARD // 128):
                            nc.tensor.matmul(
                                pos[ss][:],
                                lhsT=otn[:, 3 * SHARD + ss * 128:
                                         3 * SHARD + (ss + 1) * 128],
                                rhs=wo_sb[:, 3 * 512:4 * 512],
                                start=False, stop=True)
                            osb = pfo.tile([128, 512], F32, tag="osb",
                                           name="osb")
                            nc.vector.tensor_copy(osb[:], pos[ss][:])
                            eng = (nc.sync, nc.gpsimd, nc.scalar, nc.sync)[ss]
                            eng.dma_start(
                                out_shard[ss * 128:(ss + 1) * 128, :], osb[:])
    if hoist:
        _split_matmul_waits(nc)
    return nc


def get_program(S=4096):
    if S not in _PROGRAM_CACHE:
        _PROGRAM_CACHE[S] = build_program(S)
    return _PROGRAM_CACHE[S]


def make_in_maps(hidden_states, Wq, Wk, Wv, Wo, cos, sin, num_registers, S):
    """Host-side layout packing: X^T, per-head weight slices, the signed trig
    table (identity rotation for register tokens), bf16 Wo and identity."""
    nr = int(num_registers)
    X = np.asarray(hidden_states, dtype=np.float32).reshape(S, HIDDEN)
    XT = np.ascontiguousarray(X.T)
    Wq = np.asarray(Wq, dtype=np.float32)
    Wk = np.asarray(Wk, dtype=np.float32)
    Wv = np.asarray(Wv, dtype=np.float32)
    Wo = np.asarray(Wo, dtype=np.float32)
    cos = np.asarray(cos, dtype=np.float32)
    sin = np.asarray(sin, dtype=np.float32)

    cos_full = np.ones((S, HD), np.float32)
    sin_full = np.zeros((S, HD), np.float32)
    if nr < S:
        cos_full[nr:] = cos[:S - nr]
        sin_full[nr:] = sin[:S - nr]
    cosT = cos_full.T
    nsinlo = -sin_full.T[0:HD // 2]
    sinhi = sin_full.T[HD // 2:]
    trigA = np.concatenate([cosT, cosT], axis=0).astype(ml_dtypes.bfloat16)
    # trigB rows sit at the partition of the rope mul's SOURCE rows:
    # rows 0:32 multiply pk[0:32] (sinhi), rows 32:64 multiply pk[32:64]
    trigB = np.concatenate([sinhi, nsinlo, sinhi, nsinlo],
                           axis=0).astype(ml_dtypes.bfloat16)
    woT = np.ascontiguousarray(Wo.T).astype(ml_dtypes.bfloat16)
    identb = np.eye(128, dtype=ml_dtypes.bfloat16)
    sel2 = np.zeros((2, 128), ml_dtypes.bfloat16)
    sel2[0, 0:64] = 1
    sel2[1, 64:128] = 1

    in_maps = []
    for c in range(NCORES):
        sl = slice(c * HD, (c + 1) * HD)
        wqkT = np.ascontiguousarray(np.concatenate([Wq[sl], Wk[sl]], axis=0).T)
        wvT = np.ascontiguousarray(Wv[sl].T)
        in_maps.append({
            "xT": XT, "wqkT": wqkT, "wvT": wvT, "woT": woT, "trigA": trigA,
            "trigB": trigB, "identb": identb, "sel2": sel2,
        })
    return in_maps


def kernel(hidden_states, Wq, Wk, Wv, Wo, cos, sin, num_registers):
    hidden_states = np.asarray(hidden_states)
    B, S, H = hidden_states.shape
    assert B == 1 and H == HIDDEN
    nc = get_program(S)
    in_maps = make_in_maps(hidden_states, Wq, Wk, Wv, Wo, cos, sin,
                           num_registers, S)
    res = run_bass_kernel_spmd(nc, in_maps, list(range(NCORES)))
    shards = [res.results[c]["out_shard"] for c in range(NCORES)]
    out = np.concatenate(shards, axis=0).reshape(1, S, HIDDEN)
    return out.astype(np.float32)
